# revision 35
# baseline (speedup 1.0000x reference)
"""Multi-head attention (qkv pointwise-conv projection + softmax attention)
on 8 Trainium2 NeuronCores.

Problem shapes (hardcoded):
    x:     [B=4, D=512, L=2048] f32
    w_qkv: [3*D=1536, D=512]    f32
    out:   [B, D, L]            f32

Sharding: 2 cores per batch element; each core owns 4 of the 8 heads
(tensor-parallel on the qkv output channels). Core c -> batch c//2,
head group c%2 (heads 4*(c%2) .. 4*(c%2)+3).

Design (v3, trace-driven): the kernel is bound by the ScalarE exp floor
(128 activations x [128,1024], ~1.0us cadence = ~128us; exp exists only
on ScalarE - walrus rejects Pool-engine activations, and GPSIMD cannot
touch PSUM). Everything else is scheduled to keep ScalarE busy:
  - software-pipelined period: st(t+1) emitted first, attn@v lagged one
    period, projection fillers split into half-groups (~500ns) that fit
    the absorbable per-period PE spill
  - the NEXT block's st(0) and st(1) are emitted around this block's
    last exp, so ScalarE rolls straight through block boundaries
  - separate PSUM tag rings (st 2x2 banks, o_acc 2x1, proj 2x1) so
    transient proj tiles never wait on live accumulators; o_acc is
    copied to SBUF at block end so normalization runs fully decoupled
  - normalization: 1/den via DVE approx-reciprocal (needs a partition-0
    SBUF input), broadcast across 64 partitions via a DRAM round trip
    (no PE matmul: an early bcast matmul parks in the PE's 4-deep wait
    queue and blocks everything behind it); the last block uses a bf16
    ones-matmul broadcast instead (short latency chain for the tail)
  - input DMA split across the sync/scalar HWDGE rings + gpsimd SWDGE
    with the critical slices (wqk pair-0, x chunk 0) first; st(0) only
    waits for the first 128 k columns
  - few tile pools (each pool enter/exit costs a full engine barrier)
"""

import os
import numpy as np

B, D, L, H = 4, 512, 2048, 8
HD = D // H  # 64
N_CORES = 8
SCALE = float(D) ** -0.5

# module-level knobs for test.py; harness uses defaults
TRACE = False
LAST_RESULTS = None

_COMPILED = {}


def _build_nc():
    from contextlib import ExitStack

    import concourse.bass as bass
    import concourse.mybir as mybir
    import concourse.tile as tile
    from concourse.bacc import Bacc

    F32 = mybir.dt.float32
    F32R = mybir.dt.float32r
    BF16 = mybir.dt.bfloat16
    Exp = mybir.ActivationFunctionType.Exp

    # Bacc (not plain Bass): its finalize() runs the legalization passes that
    # split multi-wait matmuls (walrus MM struct supports only 1 sync wait).
    nc = Bacc("TRN2", target_bir_lowering=False, debug=False)
    # host pre-permuted layouts -> fully contiguous DMA descriptors
    # x: [p, lc, dc, l'] where d = dc*128+p, l = lc*512+l'
    x_d = nc.dram_tensor("x", [128, 4, 4, 512], BF16, kind="ExternalInput")
    # wT pair-major: [p, pair, dc, (q128|k128)] so the pair-0 slice (needed
    # for the first projections) is one contiguous DMA
    wqk_d = nc.dram_tensor("wqkT", [128, 2, 4, 256], BF16, kind="ExternalInput")
    wv_d = nc.dram_tensor("wvT", [128, 4, 256], BF16, kind="ExternalInput")
    out_d = nc.dram_tensor("out", [256, L], F32, kind="ExternalOutput")

    NJB = L // 128  # 16 key blocks
    NIC = L // 512  # 4 query chunks

    with ExitStack() as ctx:
        tc = ctx.enter_context(tile.TileContext(nc))
        sb = ctx.enter_context(tc.tile_pool(name="sb", bufs=1))
        ps = ctx.enter_context(tc.tile_pool(name="ps", bufs=2, space="PSUM"))
        drp = ctx.enter_context(tc.tile_pool(name="drp", bufs=4, space="DRAM"))

        # ---- PE warmup + input DMA ----
        # zero-input matmuls keep the PE busy from t~7.5us so the HAM clock
        # gate opens (1.2 -> 2.4 GHz, needs ~3us of activity) while the
        # input DMAs stream in.
        scr_sb = sb.tile([128, 512], BF16, tag="scr")
        nc.vector.memset(scr_sb[:], 0.0)
        warm_ps = ps.tile([128, 1024], F32, tag="st", name="warm")
        for _ in range(10):
            nc.tensor.matmul(warm_ps[:, 0:512], scr_sb[:, 0:128], scr_sb[:])

        wqk_sb = sb.tile([128, 2, 4, 256], BF16, tag="wqk")
        wv_sb = sb.tile([128, 4, 256], BF16, tag="wv")
        x_sb = sb.tile([128, 4, 4, 512], BF16, tag="x")
        # DMA completion is descriptor-dispatch-bound (~36ns/descriptor per
        # queue, one descriptor per partition-run). Split every critical
        # tensor across the two HWDGE rings by partition halves so the
        # first-projection inputs (wqk pair-0 + x chunk 0) land in ~half
        # the time; wv rides the (slower) gpsimd SWDGE ring.
        nc.sync.dma_start(out=wqk_sb[:, 0], in_=wqk_d[:, 0])
        nc.scalar.dma_start(out=x_sb[:, 0], in_=x_d[:, 0])
        nc.gpsimd.dma_start(out=wv_sb[:], in_=wv_d[:])
        nc.sync.dma_start(out=x_sb[:, 1], in_=x_d[:, 1])
        nc.gpsimd.dma_start(out=x_sb[:, 2], in_=x_d[:, 2])
        nc.scalar.dma_start(out=x_sb[:, 3], in_=x_d[:, 3])
        nc.sync.dma_start(out=wqk_sb[:, 1], in_=wqk_d[:, 1])

        ones_sb = sb.tile([1, 64], BF16, tag="ones")
        nc.vector.memset(ones_sb[:], 1.0)

        q_sb = [sb.tile([128, L], BF16, tag=f"q{p}", name=f"q{p}") for p in range(2)]
        k_sb = [sb.tile([128, L], BF16, tag=f"k{p}", name=f"k{p}") for p in range(2)]
        # all vt blocks in one tile (fewer sems); ones column set once
        vt_sb = sb.tile([128, NJB, 4, 65], BF16, tag="vt")
        nc.vector.memset(vt_sb[:, :, :, 64:65], 1.0)

        # ---- projection groups (PE work in the proj/bcast PSUM ring) ----
        def g_qk(p, qk, lc):
            # full 512-wide column group of q (qk=0) or k (qk=1), pair p
            def f():
                dst = q_sb[p] if qk == 0 else k_sb[p]
                pst = ps.tile([128, 512], F32, tag="proj", name="projg")
                for dc in range(4):
                    nc.tensor.matmul(
                        pst[:],
                        wqk_sb[:, p, dc, qk * 128 : (qk + 1) * 128],
                        x_sb[:, lc, dc, :],
                        start=(dc == 0),
                        stop=(dc == 3),
                    )
                nc.vector.tensor_copy(dst[:, lc * 512 : (lc + 1) * 512], pst[:])

            return f

        def g_qk_halves(p, qk, lc):
            # the same group split into 2 PE halves (2 matmuls, N=512, ~500ns)
            # so each fits the absorbable per-period PE spill (~600ns) and
            # never delays the st chain feeding ScalarE
            dst = q_sb[p] if qk == 0 else k_sb[p]
            state = {}

            def half(dh):
                def f():
                    if "ps" not in state:
                        state["ps"] = ps.tile(
                            [128, 512], F32, tag="proj", name="projh"
                        )
                    pst = state["ps"]
                    for dc in (2 * dh, 2 * dh + 1):
                        nc.tensor.matmul(
                            pst[:],
                            wqk_sb[:, p, dc, qk * 128 : (qk + 1) * 128],
                            x_sb[:, lc, dc, :],
                            start=(dc == 0),
                            stop=(dc == 3),
                        )
                    if dh == 1:
                        nc.vector.tensor_copy(
                            dst[:, lc * 512 : (lc + 1) * 512], pst[:]
                        )

                return f

            return [half(0), half(1)]

        def g_vt(jb):
            def f():
                pst = ps.tile([128, 512], F32, tag="proj", name="projv")
                for dc in range(4):
                    nc.tensor.matmul(
                        pst[:, 0:256],
                        x_sb[:, jb // 4, dc, (jb % 4) * 128 : (jb % 4 + 1) * 128],
                        wv_sb[:, dc, :],
                        start=(dc == 0),
                        stop=(dc == 3),
                    )
                nc.vector.tensor_copy(
                    vt_sb[:, jb, :, 0:64],
                    pst[:, 0:256].rearrange("par (h e) -> par h e", e=64),
                )

            return f

        def st_mms(p, ic, jb):
            st = ps.tile([128, 1024], F32, tag="st", name="st")
            for hp in range(2):
                nc.tensor.matmul(
                    st[:, hp * 512 : (hp + 1) * 512],
                    k_sb[p][hp * 64 : (hp + 1) * 64, jb * 128 : (jb + 1) * 128],
                    q_sb[p][hp * 64 : (hp + 1) * 64, ic * 512 : ic * 512 + 512],
                    start=True,
                    stop=True,
                )
            return st

        # ---- attention blocks, software-pipelined ----
        def emit_block(p, ic, fillers, lag, norm_carry, first_sts, next_st_fn, last):
            # one (head-pair, query-chunk) block: 16 exp periods.
            # Emission per period t: exp(t) | st(t+1) | carried normB |
            # fillers | attnv(t-lag). At t=15 the NEXT block's st(0) and
            # st(1) are emitted in place of st(16) so ScalarE rolls straight
            # through the boundary (st(1)'s matmuls park in the PE wait
            # queue until exp(15) frees their ring slot).
            fillers = dict(fillers)
            i0 = ic * 512

            o_ps = [
                ps.tile([65, 512], F32, tag="oacc", name="o_acc") for _ in range(2)
            ]

            def attnv(jb, se_t):
                for hp in range(2):
                    nc.tensor.matmul(
                        o_ps[hp][:],
                        vt_sb[:, jb, 2 * p + hp, :],
                        se_t[:, hp * 512 : (hp + 1) * 512],
                        start=(jb == 0),
                        stop=(jb == NJB - 1),
                    )

            pend = []
            sts = list(first_sts)
            if not sts:
                sts = [st_mms(p, ic, 0)]
            for f in fillers.get(-1, ()):
                f()
            next_firsts = []
            for jb in range(NJB):
                st_cur = sts.pop(0)
                se_t = sb.tile([128, 1024], BF16, tag="se", bufs=6)
                nc.scalar.activation(se_t[:], st_cur[:], Exp, scale=SCALE)
                if len(sts) == 0 and jb + 1 < NJB:
                    sts.append(st_mms(p, ic, jb + 1))
                elif jb == NJB - 1 and next_st_fn is not None:
                    next_firsts.append(next_st_fn(0))
                if jb in (4, 6) and norm_carry:
                    norm_carry[0 if jb == 4 else 1]()
                for f in fillers.get(jb, ()):
                    f()
                pend.append((jb, se_t))
                while len(pend) > lag:
                    j, s = pend.pop(0)
                    attnv(j, s)
            if next_st_fn is not None:
                next_firsts.append(next_st_fn(1))
            while pend:
                j, s = pend.pop(0)
                attnv(j, s)

            # normA: copy o_acc (values + den row) to SBUF, freeing the PSUM
            # slot for the next block's attnv at lag-1; 1/den via DVE approx
            # recip; broadcast 1/den across 64 partitions with a DRAM round
            # trip (no PE matmul, no PSUM pressure - an early-emitted bcast
            # matmul parks in the PE's 4-deep wait queue and blocks
            # everything behind it). normB (mult + one merged DMA) fires at
            # periods 4/6 of the NEXT block when the bounce is done.
            otmps, rbcs = [], []
            bcps = []
            for hp in range(2):
                otmp = sb.tile([65, 512], F32, tag="otmp", bufs=4)
                nc.vector.tensor_copy(otmp[:], o_ps[hp][:])
                den_sb = sb.tile([1, 512], F32, tag="den", bufs=4)
                nc.vector.tensor_copy(den_sb[:], o_ps[hp][64:65, :])
                recip = sb.tile([1, 512], F32, tag="recip", bufs=4)
                # NB: approx-recip reads garbage from PSUM on HW, and wants a
                # partition-0-based SBUF input (custom DVE op)
                nc.vector.reciprocal_approx_fast(out=recip[:], in_=den_sb[:])
                otmps.append(otmp)
                if last:
                    # tail path: PE ones-bcast (short latency chain); bf16
                    # because fp32 matmuls run at 1/4 rate in a double pass
                    recip_bf = sb.tile([1, 512], BF16, tag="recipbf", bufs=4)
                    nc.vector.tensor_copy(recip_bf[:], recip[:])
                    bc_ps = ps.tile([128, 512], F32, tag="proj", name="bcast")
                    nc.tensor.matmul(
                        bc_ps[0:64, :], ones_sb[:], recip_bf[:], start=True, stop=True
                    )
                    bcps.append(bc_ps)
                else:
                    db = drp.tile([1, 512], F32, tag="db", name="db")
                    nc.sync.dma_start(out=db[:], in_=recip[:])
                    rbc = sb.tile([64, 512], F32, tag="rbc", bufs=4)
                    nc.sync.dma_start(
                        out=rbc[:],
                        in_=bass.AP(
                            tensor=db.tensor, offset=db.offset, ap=[[0, 64], [1, 512]]
                        ),
                    )
                    rbcs.append(rbc)

            ot = sb.tile([128, 512], F32, tag="ot", bufs=2, name="ot")

            def make_normB(hp):
                def f():
                    mul_in = bcps[hp][0:64, :] if last else rbcs[hp][:]
                    nc.vector.tensor_mul(
                        ot[hp * 64 : (hp + 1) * 64, :], otmps[hp][0:64, :], mul_in
                    )
                    if hp == 1:
                        if last:
                            # tail: split across both HWDGE rings (the
                            # descriptor dispatch is the latency)
                            nc.sync.dma_start(
                                out=out_d[p * 128 : p * 128 + 64, i0 : i0 + 512],
                                in_=ot[0:64],
                            )
                            nc.scalar.dma_start(
                                out=out_d[p * 128 + 64 : p * 128 + 128, i0 : i0 + 512],
                                in_=ot[64:128],
                            )
                        else:
                            nc.sync.dma_start(
                                out=out_d[p * 128 : (p + 1) * 128, i0 : i0 + 512],
                                in_=ot[:],
                            )

                return f

            return [make_normB(0), make_normB(1)], next_firsts

        # ---- filler schedule ----
        # block 0 (p0,ic0) is demand-bound: it must produce all 15 remaining
        # vt blocks + k0 columns + q0-lc1 while running attention; full
        # groups (less ldweights overhead) front-loaded as hard as the st/
        # attnv pipeline allows. Blocks 1-3 have slack: chopped pieces only.
        # Full groups (chopping into pieces costs +33% PE in ldweights);
        # k0-lc(m) must land before the period of st(4m) (one-ahead), vt(j)
        # before attnv(j) (one-lag => same-period ok), q0-lc1 before the
        # boundary-prefetched st(0) of block 1.
        fill0 = {
            0: [g_vt(0), g_vt(1)],
            1: [g_vt(2)],
            2: [g_qk(0, 1, 1)],  # k0 lc1
            3: [g_vt(3)],
            4: [g_vt(4)],
            5: [g_qk(0, 1, 2)],  # k0 lc2
            6: [g_vt(5), g_vt(6)],
            7: [g_vt(7)],
            8: [g_vt(8)],
            9: [g_qk(0, 1, 3)],  # k0 lc3
            10: [g_vt(9), g_vt(10)],
            11: [g_vt(11)],
            12: [g_qk(0, 0, 1), g_vt(12)],  # q0 lc1
            13: [g_vt(13)],
            14: [g_vt(14), g_vt(15)],
        }
        # blocks 1-3: remaining projections as half-groups every other period
        bl_groups = [
            [(0, 0, 2), (1, 0, 0), (1, 1, 0), (1, 0, 1)],  # block 1
            [(0, 0, 3), (1, 0, 2), (1, 1, 1), (1, 1, 2)],  # block 2
            [(1, 0, 3), (1, 1, 3)],  # block 3
        ]
        fills = [fill0, {}, {}, {}, {}, {}, {}, {}]
        for bix, grps in enumerate(bl_groups, start=1):
            per = 0
            for g3 in grps:
                for h in g_qk_halves(*g3):
                    fills[bix].setdefault(per, []).append(h)
                    per += 2

        # ---- prologue: minimal path to the first exp ----
        # st(0) needs all of q0-lc0 but only k[:, 0:128]; split the k0
        # group so the first exp isn't gated on the full k copy
        g_qk(0, 0, 0)()  # q0 lc0
        k0a = ps.tile([128, 512], F32, tag="proj", name="k0a")
        for dc in range(4):
            nc.tensor.matmul(
                k0a[:, 0:128],
                wqk_sb[:, 0, dc, 128:256],
                x_sb[:, 0, dc, 0:128],
                start=(dc == 0),
                stop=(dc == 3),
            )
        nc.vector.tensor_copy(k_sb[0][:, 0:128], k0a[:, 0:128])

        def k0_rest():
            pst = ps.tile([128, 512], F32, tag="proj", name="k0b")
            for dc in range(4):
                nc.tensor.matmul(
                    pst[:, 0:384],
                    wqk_sb[:, 0, dc, 128:256],
                    x_sb[:, 0, dc, 128:512],
                    start=(dc == 0),
                    stop=(dc == 3),
                )
            nc.vector.tensor_copy(k_sb[0][:, 128:512], pst[:, 0:384])

        fill0[-1] = [k0_rest]

        blocks = [(p_, ic_) for p_ in range(2) for ic_ in range(4)]
        norm_carry = []
        first_sts = []
        for bi2, (p_, ic_) in enumerate(blocks):
            if bi2 + 1 < len(blocks):
                p3, ic3 = blocks[bi2 + 1]
                next_fn = lambda jb3, p3=p3, ic3=ic3: st_mms(p3, ic3, jb3)
            else:
                next_fn = None
            norm_carry, first_sts = emit_block(
                p_, ic_, fills[bi2], 1, norm_carry, first_sts, next_fn,
                last=(bi2 == 7),
            )
        for f in norm_carry:
            f()

    nc.finalize()
    return nc


def _get_nc():
    if "nc" not in _COMPILED:
        _COMPILED["nc"] = _build_nc()
    return _COMPILED["nc"]


def _prep_inputs(x, w_qkv):
    """Per-core input maps (host-side sharding)."""
    import ml_dtypes

    bf16 = ml_dtypes.bfloat16
    in_maps = []
    for c in range(N_CORES):
        b, g = c // 2, c % 2
        # x[b] [512, 2048] -> [p, lc, dc, l'] so every DMA descriptor is a
        # 4KB contiguous run
        xb = np.ascontiguousarray(
            x[b].reshape(4, 128, 4, 512).transpose(1, 2, 0, 3)
        ).astype(bf16)
        # w rows for this head group, transposed, pair-major:
        # wqkT[p, pair, dc, 0:128]  = q columns of head-pair `pair`
        # wqkT[p, pair, dc, 128:256] = k columns of head-pair `pair`
        wq_rows = w_qkv[256 * g : 256 * (g + 1), :]  # [256, 512]
        wk_rows = w_qkv[512 + 256 * g : 512 + 256 * (g + 1), :]  # [256, 512]
        wqT = wq_rows.T.reshape(4, 128, 2, 128)  # [dc, p, pair, o]
        wkT = wk_rows.T.reshape(4, 128, 2, 128)
        wqkT = np.ascontiguousarray(
            np.concatenate([wqT, wkT], axis=3).transpose(1, 2, 0, 3)
        ).astype(bf16)  # [p, pair, dc, 256]
        wv_rows = w_qkv[1024 + 256 * g : 1024 + 256 * (g + 1), :]  # [256, 512]
        wvT = np.ascontiguousarray(
            wv_rows.T.reshape(4, 128, 256).transpose(1, 0, 2)
        ).astype(bf16)
        in_maps.append({"x": xb, "wqkT": wqkT, "wvT": wvT})
    return in_maps


def kernel(x, w_qkv):
    global LAST_RESULTS
    from concourse.bass_utils import run_bass_kernel_spmd

    nc = _get_nc()
    in_maps = _prep_inputs(np.asarray(x), np.asarray(w_qkv))
    res = run_bass_kernel_spmd(
        nc, in_maps, core_ids=list(range(N_CORES)), trace=TRACE
    )
    LAST_RESULTS = res
    out = np.empty((B, D, L), dtype=np.float32)
    for c in range(N_CORES):
        b, g = c // 2, c % 2
        out[b, 256 * g : 256 * (g + 1), :] = res.results[c]["out"]
    return out
